# revision 3
# baseline (speedup 1.0000x reference)
"""Tensor-parallel attention kernel for 8 Trainium2 NeuronCores.

Reference computation (S=2048, B=2, H=2048, NH=16 heads, HD=128):
    q = x @ wq.T ; k = x @ wk.T ; v = x @ wv.T          (x: [S, B, H])
    per (b, head): out = softmax(q k^T / sqrt(HD)) v
    return concat_heads(out) @ wo.T                      ([S, B, H])

Sharding: tensor-parallel over heads. Core r owns heads {2r, 2r+1}:
column-parallel wq/wk/wv shards, row-parallel wo shard, ReduceScatter(add)
over the partial outputs; the host concatenates the 8 scatter slices.

On-core dataflow (everything f32/f32r; tokens ordered b-major: t = b*S + s):
  phase 1: qT/kT [256 feat, 4096 tok] = wT.T @ xT ; v [4096 tok, 256 feat]
  phase 2: per (b, h), per 512-wide q-tile: scoresT [j, q] blocks ->
           exp on ScalarE -> pT ; sums via ones-matmul (broadcast rows);
           PV matmul gives oT [feat, tok]; normalize by 1/sums on VectorE
  phase 3: partialT = oT.T-free projection: out[tok, :] += oT-slices @ woT
           -> partial [4096, 2048] in DRAM -> chunked ReduceScatter
"""

import numpy as np

S, B, H = 2048, 2, 2048
NH, HD = 16, 128
N_CORES = 8
HPC = NH // N_CORES          # heads per core (2)
FPC = HPC * HD               # features per core (256)
NT = S * B                   # tokens (4096)
SCALE = HD ** -0.5
KT = H // 128                # contraction tiles in phase 1 (16)
NB = 512                     # token block width in phase 1
QT = 512                     # q-tile width in phase 2
RS_CHUNKS = 4                # ReduceScatter chunks
CHUNK = NT // RS_CHUNKS      # tokens per chunk (1024)
SLICE = CHUNK // N_CORES     # rows a core receives per chunk (128)


def _build():
    import concourse.mybir as mybir
    import concourse.tile as tile
    from concourse import bacc

    F32 = mybir.dt.float32
    F32R = mybir.dt.float32r
    BF16 = mybir.dt.bfloat16
    Exp = mybir.ActivationFunctionType.Exp
    Copy = mybir.ActivationFunctionType.Copy

    nc = bacc.Bacc(None, target_bir_lowering=False, num_devices=N_CORES)

    xT = nc.dram_tensor("xT", [H, NT], F32R, kind="ExternalInput")
    wqT = nc.dram_tensor("wqT", [H, FPC], F32R, kind="ExternalInput")
    wkT = nc.dram_tensor("wkT", [H, FPC], F32R, kind="ExternalInput")
    wvT = nc.dram_tensor("wvT", [H, FPC], F32R, kind="ExternalInput")
    woT = nc.dram_tensor("woT", [FPC, H], F32, kind="ExternalInput")
    out = nc.dram_tensor(
        "out", [RS_CHUNKS * SLICE, H], F32, kind="ExternalOutput"
    )

    with tile.TileContext(nc) as tc:
        with (
            tc.tile_pool(name="qk_res", bufs=1) as qk_res,
            tc.tile_pool(name="v_res", bufs=32) as v_res,
            tc.tile_pool(name="o_res", bufs=1) as o_res,
            tc.tile_pool(name="const", bufs=1) as const,
            tc.tile_pool(name="dram", bufs=1, space="DRAM") as dram,
        ):
            ones_f = const.tile([128, 128], F32)
            nc.vector.memset(ones_f[:], 1.0)
            ones = const.tile([128, 128], F32R)
            nc.vector.tensor_copy(ones[:], ones_f[:])

            qhat = [qk_res.tile([128, NT], F32R, tag=f"q{m}", name=f"qhat{m}") for m in range(2)]
            khat = [qk_res.tile([128, NT], F32R, tag=f"k{m}", name=f"khat{m}") for m in range(2)]
            vsb = [v_res.tile([128, FPC], F32R, tag="v", name=f"vsb{i}") for i in range(NT // 128)]
            ohat = [o_res.tile([128, NT], BF16, tag=f"o{m}", name=f"ohat{m}") for m in range(2)]

            partial = dram.tile([NT, H], F32)

            # ---------------- phase 1: QKV projections ----------------
            with (
                tc.tile_pool(name="w_p1", bufs=16) as w_p1,
                tc.tile_pool(name="x_p1", bufs=20) as x_p1,
                tc.tile_pool(name="ps_qk", bufs=5, space="PSUM") as ps_qk,
                tc.tile_pool(name="ps_v", bufs=3, space="PSUM") as ps_v,
            ):
                wq_t, wk_t, wv_t = [], [], []
                for kt in range(KT):
                    for (lst, src, tag) in (
                        (wq_t, wqT, "wq"),
                        (wk_t, wkT, "wk"),
                        (wv_t, wvT, "wv"),
                    ):
                        t = w_p1.tile([128, FPC], F32R, tag=tag)
                        nc.sync.dma_start(t[:], src[kt * 128 : (kt + 1) * 128, :])
                        lst.append(t)

                for nb in range(NT // NB):
                    xt = []
                    for kt in range(KT):
                        t = x_p1.tile([128, NB], F32R, tag="x")
                        nc.sync.dma_start(
                            t[:], xT[kt * 128 : (kt + 1) * 128, nb * NB : (nb + 1) * NB]
                        )
                        xt.append(t)
                    # qT / kT m-tiles: [128 feat, NB tok]
                    for dest, wt in ((qhat, wq_t), (khat, wk_t)):
                        for m in range(2):
                            ps = ps_qk.tile([128, NB], F32, tag="qk")
                            for kt in range(KT):
                                nc.tensor.matmul(
                                    ps[:],
                                    wt[kt][:, m * 128 : (m + 1) * 128],
                                    xt[kt][:],
                                    start=(kt == 0),
                                    stop=(kt == KT - 1),
                                )
                            nc.scalar.activation(
                                dest[m][:, nb * NB : (nb + 1) * NB], ps[:], Copy
                            )
                    # v natural: [128 tok, FPC]
                    for sub in range(NB // 128):
                        ps = ps_v.tile([128, FPC], F32, tag="v")
                        for kt in range(KT):
                            nc.tensor.matmul(
                                ps[:],
                                xt[kt][:, sub * 128 : (sub + 1) * 128],
                                wv_t[kt][:],
                                start=(kt == 0),
                                stop=(kt == KT - 1),
                            )
                        nc.vector.tensor_copy(vsb[nb * 4 + sub][:], ps[:])

            # ---------------- phase 2: attention ----------------
            with (
                tc.tile_pool(name="p_p2", bufs=24) as p_p2,
                tc.tile_pool(name="r_p2", bufs=4) as r_p2,
                tc.tile_pool(name="ps_sc", bufs=3, space="PSUM") as ps_sc,
                tc.tile_pool(name="ps_pv", bufs=2, space="PSUM") as ps_pv,
                tc.tile_pool(name="ps_sum", bufs=2, space="PSUM") as ps_sum,
            ):
                JB = S // 128  # 16 key blocks per (b, h)
                for b in range(B):
                    for h in range(HPC):
                        q_bh = qhat[h][:, b * S : (b + 1) * S]
                        k_bh = khat[h][:, b * S : (b + 1) * S]
                        for qt in range(S // QT):
                            pv_ps = ps_pv.tile([128, QT], F32, tag="pv")
                            sum_ps = ps_sum.tile([128, QT], F32, tag="sum")
                            for jb in range(JB):
                                sc_ps = ps_sc.tile([128, QT], F32, tag="sc")
                                nc.tensor.matmul(
                                    sc_ps[:],
                                    k_bh[:, jb * 128 : (jb + 1) * 128],
                                    q_bh[:, qt * QT : (qt + 1) * QT],
                                    start=True,
                                    stop=True,
                                )
                                pT = p_p2.tile([128, QT], F32R, tag="p")
                                nc.scalar.activation(pT[:], sc_ps[:], Exp, scale=SCALE)
                                nc.tensor.matmul(
                                    sum_ps[:],
                                    ones[:],
                                    pT[:],
                                    start=(jb == 0),
                                    stop=(jb == JB - 1),
                                )
                                nc.tensor.matmul(
                                    pv_ps[:],
                                    vsb[b * JB + jb][:, h * 128 : (h + 1) * 128],
                                    pT[:],
                                    start=(jb == 0),
                                    stop=(jb == JB - 1),
                                )
                            recip = r_p2.tile([128, QT], F32, tag="r")
                            nc.vector.reciprocal(recip[:], sum_ps[:])
                            nc.vector.tensor_mul(
                                ohat[h][:, b * S + qt * QT : b * S + (qt + 1) * QT],
                                pv_ps[:],
                                recip[:],
                            )

            # ---------------- phase 3: output projection + RS ----------------
            with (
                tc.tile_pool(name="wo_p3", bufs=2) as wo_p3,
                tc.tile_pool(name="out_p3", bufs=4) as out_p3,
                tc.tile_pool(name="ps_o", bufs=6, space="PSUM") as ps_o,
                tc.tile_pool(name="rs_dram", bufs=4, space="DRAM") as rs_dram,
            ):
                wo_t = []
                for kt in range(2):
                    t = wo_p3.tile([128, H], BF16, tag="wo")
                    nc.gpsimd.dma_start(t[:], woT[kt * 128 : (kt + 1) * 128, :])
                    wo_t.append(t)

                rs_outs = []
                for c in range(RS_CHUNKS):
                    for tb in range(CHUNK // 128):
                        tok = c * CHUNK + tb * 128
                        for nt in range(H // 512):
                            ps = ps_o.tile([128, 512], F32, tag="o")
                            for kt in range(2):
                                nc.tensor.matmul(
                                    ps[:],
                                    ohat[kt][:, tok : tok + 128],
                                    wo_t[kt][:, nt * 512 : (nt + 1) * 512],
                                    start=(kt == 0),
                                    stop=(kt == 1),
                                )
                            ev = out_p3.tile([128, 512], F32, tag="ev")
                            nc.scalar.activation(ev[:], ps[:], Copy)
                            nc.sync.dma_start(
                                partial[tok : tok + 128, nt * 512 : (nt + 1) * 512],
                                ev[:],
                            )
                    rs_c = rs_dram.tile([SLICE, H], F32, tag=f"rs{c}")
                    nc.gpsimd.collective_compute(
                        "ReduceScatter",
                        mybir.AluOpType.add,
                        replica_groups=[list(range(N_CORES))],
                        ins=[partial[c * CHUNK : (c + 1) * CHUNK, :].opt()],
                        outs=[rs_c[:].opt()],
                    )
                    rs_outs.append(rs_c)
                for c in range(RS_CHUNKS):
                    nc.sync.dma_start(
                        out[c * SLICE : (c + 1) * SLICE, :], rs_outs[c][:]
                    )
    nc.compile()
    return nc


_NC_CACHE = None


def _get_nc():
    global _NC_CACHE
    if _NC_CACHE is None:
        _NC_CACHE = _build()
    return _NC_CACHE


def kernel(x, wq, wk, wv, wo):
    from concourse.bass_utils import run_bass_kernel_spmd

    x = np.asarray(x, dtype=np.float32)
    wq = np.asarray(wq, dtype=np.float32)
    wk = np.asarray(wk, dtype=np.float32)
    wv = np.asarray(wv, dtype=np.float32)
    wo = np.asarray(wo, dtype=np.float32)

    # tokens b-major: t = b*S + s
    xT = np.ascontiguousarray(x.transpose(2, 1, 0).reshape(H, NT))

    in_maps = []
    for r in range(N_CORES):
        sl = slice(r * FPC, (r + 1) * FPC)
        in_maps.append(
            {
                "xT": xT,
                "wqT": np.ascontiguousarray(wq[sl, :].T),
                "wkT": np.ascontiguousarray(wk[sl, :].T),
                "wvT": np.ascontiguousarray(wv[sl, :].T),
                "woT": np.ascontiguousarray(wo[:, sl].T),
            }
        )

    res = run_bass_kernel_spmd(_get_nc(), in_maps, list(range(N_CORES)))

    out_bs = np.empty((NT, H), dtype=np.float32)
    for r in range(N_CORES):
        o = res.results[r]["out"]
        for c in range(RS_CHUNKS):
            tok = c * CHUNK + r * SLICE
            out_bs[tok : tok + SLICE] = o[c * SLICE : (c + 1) * SLICE]
    return np.ascontiguousarray(out_bs.reshape(B, S, H).transpose(1, 0, 2))
